# revision 1
# baseline (speedup 1.0000x reference)
"""Trainium2 Bass kernel for nn_EncoderMemNN_14929306321427 (MemNN encoder).

Math (see reference.py): story (M=256, B=16, S=64) token ids; C (4, V, 128)
embedding tables. Per hop h: m_A = sum_S C[h][s], prob = softmax_M(m_A @ u),
m_C = sum_S C[h+1][s], u += prob @ m_C. u starts at 0, so hop-0's softmax is
uniform: C[0] is never needed and u after hop 0 is mean_M(E1).

Strategy: data-parallel over batch (2 rows/core, 8 cores, no collectives).
Host fuses tables 1..3 into ccat[V+1, 384] fp16 (row V = 0) so each token is
ONE 768B dma_gather row. dma_gather indices are int16, so tokens are split at
32768: call A gathers low tokens from the table base, call B gathers high
tokens from a +32768 row view; slots not owned by a call point at an all-zero
row (PAD row 0 / appended row V), which adds 0 to the sum. Tokens are sorted
within each sentence and sentences are nlow-balanced across groups so the two
calls cover disjoint near-minimal slot ranges (~6% filler). The sentence-sum
runs on the PE as identity-matmul accumulation into PSUM (fp32-exact), then a
tiny PE/ACT/DVE attention pipeline computes the 3 hops.
"""

import numpy as np

HOPS = 3
V = 50257
D = 128
M = 256
B = 16
S = 64
NCORES = 8
BL = B // NCORES            # batch rows per core
NS = BL * M                 # sentences per core
P = 128
NG = NS // P                # sentence groups of 128
DCAT = HOPS * D             # 384 = fused row [C1|C2|C3]
NEG = -1e30
VSPLIT = 32768
ZHIGH = V - VSPLIT          # index of appended zero row within the high view

_CACHE = {}


def _consts():
    ident = np.eye(P, dtype=np.float32)
    i2 = np.eye(2, dtype=np.float32)
    identg = np.eye(P, dtype=np.float16)
    return {"ident": ident, "i2": i2, "identg": identg}


def build(KA, KB, do_compile=True):
    """KA/KB: per-group slot counts for the low/high gather calls."""
    from concourse import bacc, mybir, tile

    f32 = mybir.dt.float32
    f16 = mybir.dt.float16
    i16 = mybir.dt.int16
    Alu = mybir.AluOpType
    Act = mybir.ActivationFunctionType
    Ax = mybir.AxisListType

    nc = bacc.Bacc(num_swdge_queues=2)
    ccat_d = nc.declare_dram_parameter("ccat", [V + 1, DCAT], f16, isOutput=False)
    idx_d = {}
    for g in range(NG):
        idx_d[g, "a"] = nc.declare_dram_parameter(
            f"idxa{g}", [P, P * KA[g] // 16], i16, isOutput=False)
        idx_d[g, "b"] = nc.declare_dram_parameter(
            f"idxb{g}", [P, P * KB[g] // 16], i16, isOutput=False)
    ident_d = nc.declare_dram_parameter("ident", [P, P], f32, isOutput=False)
    identg_d = nc.declare_dram_parameter("identg", [P, P], f16, isOutput=False)
    i2_d = nc.declare_dram_parameter("i2", [2, 2], f32, isOutput=False)
    sel_d = nc.declare_dram_parameter("sel", [P, NG * 2], f32, isOutput=False)
    mneg_d = nc.declare_dram_parameter("mneg", [BL, BL * M], f32, isOutput=False)
    out_d = nc.declare_dram_parameter("out", [BL, D], f32, isOutput=True)

    with tile.TileContext(nc) as tc:
        with (
            tc.tile_pool(name="const", bufs=1) as cpool,
            tc.tile_pool(name="gather", bufs=2) as gpool,
            tc.tile_pool(name="work", bufs=2) as wpool,
            tc.tile_pool(name="ps_e", bufs=2, space="PSUM") as ps_e,
            tc.tile_pool(name="ps_t", bufs=2, space="PSUM") as ps_t,
            tc.tile_pool(name="ps_col", bufs=1, space="PSUM") as ps_col,
            tc.tile_pool(name="ps_mm", bufs=1, space="PSUM") as ps_mm,
        ):
            idx_sb = {}
            for g in range(NG):
                for h in ("a", "b"):
                    t = cpool.tile(list(idx_d[g, h].shape), i16, tag=f"idx{h}{g}")
                    nc.sync.dma_start(out=t[:], in_=idx_d[g, h][:])
                    idx_sb[g, h] = t
            ident = cpool.tile([P, P], f32)
            nc.sync.dma_start(out=ident[:], in_=ident_d[:])
            identg = cpool.tile([P, P], f16)
            nc.sync.dma_start(out=identg[:], in_=identg_d[:])
            i2 = cpool.tile([2, 2], f32)
            nc.sync.dma_start(out=i2[:], in_=i2_d[:])
            sel = cpool.tile([P, NG * 2], f32)
            nc.sync.dma_start(out=sel[:], in_=sel_d[:])
            mneg = cpool.tile([BL, BL * M], f32)
            nc.sync.dma_start(out=mneg[:], in_=mneg_d[:])

            # ---- gather + sentence-sum: E_all[p, g*DCAT+d] = sum_S ccat[tok]
            E_all = cpool.tile([P, NG * DCAT], f32)
            for g in range(NG):
                gta = gpool.tile([P, KA[g], DCAT], f16, tag="gta")
                nc.gpsimd.dma_gather(
                    out_ap=gta[:], in_ap=ccat_d[:], idxs_ap=idx_sb[g, "a"][:],
                    num_idxs=P * KA[g], num_idxs_reg=P * KA[g],
                    elem_size=DCAT, single_packet=False,
                )
                gtb = gpool.tile([P, KB[g], DCAT], f16, tag="gtb")
                nc.gpsimd.dma_gather(
                    out_ap=gtb[:], in_ap=ccat_d[VSPLIT:, :], idxs_ap=idx_sb[g, "b"][:],
                    num_idxs=P * KB[g], num_idxs_reg=P * KB[g],
                    elem_size=DCAT, single_packet=False, queue_num=1,
                )
                eps = ps_e.tile([P, DCAT], f32, tag="eacc")
                tot = KA[g] + KB[g]
                nmm = 0
                for gt, kk in ((gta, KA[g]), (gtb, KB[g])):
                    for r in range(kk):
                        nc.tensor.matmul(
                            out=eps[:], lhsT=identg[:], rhs=gt[:, r, :],
                            start=(nmm == 0), stop=(nmm == tot - 1),
                        )
                        nmm += 1
                nc.vector.tensor_copy(out=E_all[:, g * DCAT:(g + 1) * DCAT], in_=eps[:])

            # transposed E1/E2 for the logits matmuls (filled per group):
            # F_t[:, g*P:(g+1)*P] = (E_t block of group g).T   [d, sentence]
            F1 = cpool.tile([P, NS], f32)
            F2 = cpool.tile([P, NS], f32)
            us = ps_mm.tile([BL, DCAT], f32, tag="usum")
            for g in range(NG):
                for t, F in ((0, F1), (1, F2)):
                    tp = ps_t.tile([P, P], f32, tag="tp")
                    nc.tensor.transpose(
                        out=tp[:],
                        in_=E_all[:, g * DCAT + t * D: g * DCAT + t * D + D],
                        identity=ident[:],
                    )
                    nc.scalar.copy(out=F[:, g * P:(g + 1) * P], in_=tp[:])
                # hop 0: u = mean_M E1[b] (softmax of zero logits is uniform);
                # sel col b marks this group's sentences owned by batch row b
                nc.tensor.matmul(
                    out=us[:], lhsT=sel[:, g * 2:(g + 1) * 2],
                    rhs=E_all[:, g * DCAT:(g + 1) * DCAT],
                    start=(g == 0), stop=(g == NG - 1),
                )
            u = wpool.tile([BL, D], f32, tag="u0")
            nc.scalar.activation(
                out=u[:], in_=us[0:BL, 0:D], func=Act.Copy, scale=1.0 / M
            )

            # ---- hops 1..2
            for hop in (1, 2):
                F = F1 if hop == 1 else F2
                # u as columns: uc[d, b] = u[b, d]
                uc_ps = ps_col.tile([P, BL], f32, tag="colT")
                nc.tensor.matmul(out=uc_ps[:], lhsT=u[:], rhs=i2[:], start=True, stop=True)
                uc = wpool.tile([P, BL], f32, tag="uc")
                nc.scalar.copy(out=uc[:], in_=uc_ps[:])
                # logits[b, b'*M+m] = sum_d u[b,d] * E_hop[b',m,d]; mask kills b!=b'
                lg_ps = ps_mm.tile([BL, NS], f32, tag="lg")
                nc.tensor.matmul(out=lg_ps[:], lhsT=uc[:], rhs=F[:], start=True, stop=True)
                lgm = wpool.tile([BL, NS], f32, tag="lgm")
                nc.vector.scalar_tensor_tensor(
                    out=lgm[:], in0=lg_ps[:], scalar=1.0, in1=mneg[:],
                    op0=Alu.mult, op1=Alu.add,
                )
                nmax = wpool.tile([BL, 1], f32, tag="nmax")
                nc.vector.tensor_reduce(
                    out=nmax[:], in_=lgm[:], axis=Ax.X, op=Alu.max, negate=True
                )
                pe = wpool.tile([BL, NS], f32, tag="pe")
                den = wpool.tile([BL, 1], f32, tag="den")
                nc.scalar.activation(
                    out=pe[:], in_=lgm[:], func=Act.Exp, bias=nmax[:], scale=1.0,
                    accum_out=den[:],
                )
                rden = wpool.tile([BL, 1], f32, tag="rden")
                nc.vector.reciprocal(out=rden[:], in_=den[:])
                # o[b, d] = sum_m pe[b, m] * E_{hop+1}[b, m, d]  (normalized below)
                o_ps = ps_mm.tile([BL, D], f32, tag="o")
                for g in range(NG):
                    pt_ps = ps_col.tile([P, BL], f32, tag="colT")
                    nc.tensor.matmul(
                        out=pt_ps[:], lhsT=pe[:, g * P:(g + 1) * P], rhs=i2[:],
                        start=True, stop=True,
                    )
                    ptsb = wpool.tile([P, BL], f32, tag="ptsb")
                    nc.scalar.copy(out=ptsb[:], in_=pt_ps[:])
                    nc.tensor.matmul(
                        out=o_ps[:], lhsT=ptsb[:],
                        rhs=E_all[:, g * DCAT + hop * D: g * DCAT + hop * D + D],
                        start=(g == 0), stop=(g == NG - 1),
                    )
                # u <- u + o / den
                u2 = wpool.tile([BL, D], f32, tag=f"u{hop}")
                nc.vector.scalar_tensor_tensor(
                    out=u2[:], in0=o_ps[:], scalar=rden[:], in1=u[:],
                    op0=Alu.mult, op1=Alu.add,
                )
                u = u2

            nc.sync.dma_start(out=out_d[:], in_=u[:])
    if do_compile:
        nc.compile()
    return nc


def _wrap16(idx):
    """flat [n] int16 -> SBUF layout [128, n//16]: value i at [i%16, i//16],
    replicated to the 8 16-partition groups the Q7 cores read."""
    n = idx.shape[0]
    w = np.zeros((16, n // 16), np.int16)
    w[np.arange(n) % 16, np.arange(n) // 16] = idx
    return np.tile(w, (8, 1))


def prep_inputs(story, C):
    """Host-side: fused fp16 table, sorted/balanced per-core index layouts."""
    story = np.asarray(story)
    C = np.asarray(C, dtype=np.float32)
    s = story.transpose(1, 0, 2).astype(np.int32)       # (B, M, S)
    ccat = np.zeros((V + 1, DCAT), np.float16)
    ccat[:V] = np.concatenate([C[1], C[2], C[3]], axis=1).astype(np.float16)

    # per core: sort tokens in each sentence (low vocab first) and balance
    # sentences across the NG groups by nlow, mixing batch rows freely; the
    # uploaded sel/mneg tensors encode each sentence's batch-row ownership
    sorted_toks = []                                     # [core][g] -> (P, S)
    nlows = []                                           # [core][g] -> (P,)
    owners = []                                          # [core][g] -> (P,) batch row
    for i in range(NCORES):
        blk = s[i * BL:(i + 1) * BL].reshape(NS, S)      # (NS, S)
        own = np.repeat(np.arange(BL), M)                # (NS,)
        nlow = (blk < VSPLIT).sum(1)
        order = np.argsort(nlow, kind="stable")
        st_c, nl_c, ow_c = [], [], []
        for q in ((0, 3, 1, 2) if NG == 4 else range(NG)):
            pick = order[q * P:(q + 1) * P]
            st_c.append(np.sort(blk[pick], axis=1))
            nl_c.append(nlow[pick])
            ow_c.append(own[pick])
        sorted_toks.append(st_c)
        nlows.append(nl_c)
        owners.append(ow_c)

    KA = tuple(int(max(nlows[i][g].max() for i in range(NCORES))) for g in range(NG))
    KB = tuple(S - int(min(nlows[i][g].min() for i in range(NCORES))) for g in range(NG))

    consts = _consts()
    in_maps = []
    for i in range(NCORES):
        m = {"ccat": ccat, **consts}
        sel = np.zeros((P, NG * 2), np.float32)
        mneg = np.full((BL, BL * M), NEG, np.float32)
        for g in range(NG):
            sel[np.arange(P), g * 2 + owners[i][g]] = 1.0
            mneg[owners[i][g], g * P + np.arange(P)] = 0.0
        m["sel"] = sel
        m["mneg"] = mneg
        for g in range(NG):
            toks = sorted_toks[i][g]                     # (P, S) sorted
            nlow = nlows[i][g]                           # (P,)
            ka, kb = KA[g], KB[g]
            ks = np.arange(ka)[:, None]                  # slot k -> row k*128+p
            low = np.where(ks < nlow[None, :], toks.T[:ka], 0).astype(np.int16)
            m[f"idxa{g}"] = _wrap16(low.reshape(-1))
            k0 = S - kb
            ksb = (k0 + np.arange(kb))[:, None]
            high = np.where(
                ksb >= nlow[None, :],
                toks.T[k0:].astype(np.int64) - VSPLIT,
                ZHIGH,
            ).astype(np.int16)
            m[f"idxb{g}"] = _wrap16(high.reshape(-1))
        in_maps.append(m)
    return in_maps, KA, KB


def run(in_maps, KA, KB, trace=False, **kwargs):
    from concourse.bass_utils import run_bass_kernel_spmd

    key = (KA, KB)
    if key not in _CACHE:
        _CACHE[key] = build(KA, KB)
    nc = _CACHE[key]
    res = run_bass_kernel_spmd(
        nc, in_maps, core_ids=list(range(NCORES)), trace=trace, **kwargs
    )
    out = np.concatenate([r["out"] for r in res.results], axis=0)
    return out, res


def kernel(story, C):
    in_maps, KA, KB = prep_inputs(story, C)
    out, _ = run(in_maps, KA, KB)
    return out.astype(np.float32)



# revision 8
# speedup vs baseline: 1.2823x; 1.2823x over previous
"""Trainium2 Bass kernel for nn_EncoderMemNN_14929306321427 (MemNN encoder).

Math (see reference.py): story (M=256, B=16, S=64) token ids; C (4, V, 128)
embedding tables. Per hop h: m_A = sum_S C[h][s], prob = softmax_M(m_A @ u),
m_C = sum_S C[h+1][s], u += prob @ m_C. u starts at 0, so hop-0's softmax is
uniform: C[0] is never needed and u after hop 0 is mean_M(E1).

Strategy: data-parallel over batch (2 rows/core, 8 cores, no collectives).
Host fuses tables 1..3 into ccat[V+1, 384] fp16 (row V = 0) so each token is
ONE 768B dma_gather row. dma_gather indices are int16, so tokens are split at
32768: call A gathers low tokens from the table base, call B gathers high
tokens from a +32768 row view; slots not owned by a call point at an all-zero
row (PAD row 0 / appended row V), which adds 0 to the sum. Tokens are sorted
within each sentence and sentences are nlow-balanced across groups so the two
calls cover disjoint near-minimal slot ranges.

v2: gathers are split into ~24-slot chunks issued round-robin across 4 SWDGE
queues so the DMA engines stay continuously fed and the PE consumes chunks as
they land (instead of 8 big serialized calls). The attention tail keeps all
softmax state slot-major (logits computed transposed via F = E^T), which
replaces the per-group transpose->copy->matmul chains with a handful of tiny
matmuls: exp -> mask -> den (ones-matmul) -> weighted sum, ~2x3us instead of
~2x11us. The hop-0 state u1 = mean_M(E1) is accumulated per group in both
row- and column-major in parallel.
"""

import numpy as np

HOPS = 3
V = 50257
D = 128
M = 256
B = 16
S = 64
NCORES = 8
BL = B // NCORES            # batch rows per core
NS = BL * M                 # sentences per core
P = 128
NG = NS // P                # sentence groups of 128
DCAT = HOPS * D             # 384 = fused row [C1|C2|C3]
VSPLIT = 32768
ZHIGH = V - VSPLIT          # index of appended zero row within the high view
CH = 24                     # max gather-chunk slots
NQ = 4                      # SWDGE queues
GB = 6                      # gather chunk buffers in flight

# const blob column offsets (f32, [128, BLOBC])
OFF_IDENT = 0
OFF_SEL = 128               # sel[p, g*2+b] = 1 if sentence p of group g owned by b
OFF_I2 = 136                # 2x2 identity in rows 0..1
OFF_ONES = 138              # all-ones column
OFF_SEL2 = 139              # sel2[g*2+b, b] = 1 in rows 0..7
BLOBC = 141

_CACHE = {}


def _chunk_plan(k):
    """Split k slots into balanced chunks of at most CH."""
    if k == 0:
        return []
    n = -(-k // CH)
    base, rem = divmod(k, n)
    sizes = [base + (1 if i < rem else 0) for i in range(n)]
    out, off = [], 0
    for s in sizes:
        out.append((off, s))
        off += s
    return out


def build(KA, KB, do_compile=True):
    """KA/KB: per-group slot counts for the low/high gather calls."""
    from concourse import bacc, mybir, tile

    f32 = mybir.dt.float32
    f16 = mybir.dt.float16
    i16 = mybir.dt.int16
    Alu = mybir.AluOpType
    Act = mybir.ActivationFunctionType

    nc = bacc.Bacc(num_swdge_queues=NQ)
    ccat_d = nc.declare_dram_parameter("ccat", [V + 1, DCAT], f16, isOutput=False)
    idx_d = [
        nc.declare_dram_parameter(
            f"idx{g}", [P, 8 * (KA[g] + KB[g])], i16, isOutput=False)
        for g in range(NG)
    ]
    blob_d = nc.declare_dram_parameter("blob", [P, BLOBC], f32, isOutput=False)
    identg_d = nc.declare_dram_parameter("identg", [P, P], f16, isOutput=False)
    out_d = nc.declare_dram_parameter("out", [BL, D], f32, isOutput=True)

    with tile.TileContext(nc) as tc:
        with (
            tc.tile_pool(name="const", bufs=1) as cpool,
            tc.tile_pool(name="gather", bufs=GB) as gpool,
            tc.tile_pool(name="work", bufs=2) as wpool,
            tc.tile_pool(name="ps_e", bufs=2, space="PSUM") as ps_e,
            tc.tile_pool(name="ps_t", bufs=2, space="PSUM") as ps_t,
            tc.tile_pool(name="ps_us", bufs=1, space="PSUM") as ps_us,
            tc.tile_pool(name="ps_at", bufs=2, space="PSUM") as ps_at_pool,
        ):
            idx_sb = []
            for g in range(NG):
                t = cpool.tile(list(idx_d[g].shape), i16, tag=f"idx{g}")
                nc.sync.dma_start(out=t[:], in_=idx_d[g][:])
                idx_sb.append(t)
            blob = cpool.tile([P, BLOBC], f32)
            nc.sync.dma_start(out=blob[:], in_=blob_d[:])
            identg = cpool.tile([P, P], f16)
            nc.sync.dma_start(out=identg[:], in_=identg_d[:])

            ident = blob[:, OFF_IDENT:OFF_IDENT + P]
            sel = blob[:, OFF_SEL:OFF_SEL + NG * BL]
            i2 = blob[0:BL, OFF_I2:OFF_I2 + BL]
            ones = blob[:, OFF_ONES:OFF_ONES + 1]
            sel2 = blob[0:NG * BL, OFF_SEL2:OFF_SEL2 + BL]

            # ---- gather + sentence-sum: E_all[p, g*DCAT+d] = sum_S ccat[tok]
            # plus per group: F1/F2 = E1^T/E2^T and u1 accumulation both ways
            E_all = cpool.tile([P, NG * DCAT], f32)
            F1 = cpool.tile([P, NS], f32)
            F2 = cpool.tile([P, NS], f32)
            # two PSUM banks for the u1 accumulators (one open accumulation
            # chain per bank); their slack space is reused by the attention
            # phase for the tiny den/uc tiles once u1 has been consumed
            usbk = ps_us.tile([P, 512], f32, tag="us")
            ustbk = ps_us.tile([P, 512], f32, tag="ust")
            us_ps = usbk[0:BL, 0:D]
            ust_ps = ustbk[:, 0:BL]
            qctr = 0
            for g in range(NG):
                eps = ps_e.tile([P, DCAT], f32, tag="eacc")
                tot = KA[g] + KB[g]
                nmm = 0
                for K, base_off, src in (
                    (KA[g], 0, ccat_d[:]),
                    (KB[g], 8 * KA[g], ccat_d[VSPLIT:, :]),
                ):
                    for off, csz in _chunk_plan(K):
                        gt = gpool.tile([P, CH, DCAT], f16, tag="ch")
                        nc.gpsimd.dma_gather(
                            out_ap=gt[:, :csz, :], in_ap=src,
                            idxs_ap=idx_sb[g][:, base_off + 8 * off:
                                              base_off + 8 * (off + csz)],
                            num_idxs=P * csz, num_idxs_reg=P * csz,
                            elem_size=DCAT, single_packet=False,
                            queue_num=qctr % NQ,
                        )
                        qctr += 1
                        for r in range(csz):
                            nc.tensor.matmul(
                                out=eps[:], lhsT=identg[:], rhs=gt[:, r, :],
                                start=(nmm == 0), stop=(nmm == tot - 1),
                            )
                            nmm += 1
                nc.vector.tensor_copy(out=E_all[:, g * DCAT:(g + 1) * DCAT], in_=eps[:])
                for t, F in ((0, F1), (1, F2)):
                    tp = ps_t.tile([P, P], f32, tag="tp")
                    nc.tensor.transpose(
                        out=tp[:],
                        in_=E_all[:, g * DCAT + t * D: g * DCAT + t * D + D],
                        identity=ident,
                    )
                    nc.scalar.copy(out=F[:, g * P:(g + 1) * P], in_=tp[:])
                # hop 0: u1 = mean_M E1 (softmax of zero logits is uniform),
                # accumulated row-major (us) and col-major (ust) in parallel
                nc.tensor.matmul(
                    out=us_ps[:], lhsT=sel[:, 2 * g:2 * g + 2],
                    rhs=E_all[:, g * DCAT:g * DCAT + D],
                    start=(g == 0), stop=(g == NG - 1),
                )
                nc.tensor.matmul(
                    out=ust_ps[:], lhsT=E_all[:, g * DCAT:g * DCAT + D],
                    rhs=sel[:, 2 * g:2 * g + 2],
                    start=(g == 0), stop=(g == NG - 1),
                )

            u = wpool.tile([BL, D], f32, tag="u0")
            nc.scalar.activation(out=u[:], in_=us_ps[:], func=Act.Copy, scale=1.0 / M)
            uc = wpool.tile([P, BL], f32, tag="uc0")
            nc.scalar.activation(out=uc[:], in_=ust_ps[:], func=Act.Copy, scale=1.0 / M)

            # ---- hops 1..2, all slot-major: logits lgT[slot, g*2+b]
            for hop in (1, 2):
                F = F1 if hop == 1 else F2
                # per-hop PSUM bank for logits + o; tiny tiles live in the
                # retired u1 banks (each bank has one open chain at a time)
                at = ps_at_pool.tile([P, 512], f32, tag="at")
                lgT = at[:, 0:NG * BL]
                o_ps = at[0:BL, 16:16 + D]
                den8 = ustbk[0:NG * BL, 100 + 10 * hop:101 + 10 * hop]
                dsum = usbk[0:BL, 200 + 10 * hop:201 + 10 * hop]
                ucps = ustbk[:, 200 + 10 * hop:202 + 10 * hop]
                for g in range(NG):
                    nc.tensor.matmul(
                        out=lgT[:, 2 * g:2 * g + 2], lhsT=F[:, g * P:(g + 1) * P],
                        rhs=uc[:], start=True, stop=True,
                    )
                # exp (no max-sub: |logits| <~ 6 by construction), then mask
                # to the owning batch row (sel is exactly the 0/1 ownership)
                pe_raw = wpool.tile([P, NG * BL], f32, tag="praw")
                nc.scalar.activation(out=pe_raw[:], in_=lgT, func=Act.Exp)
                pe_t = wpool.tile([P, NG * BL], f32, tag="pet")
                nc.vector.scalar_tensor_tensor(
                    out=pe_t[:], in0=pe_raw[:], scalar=1.0, in1=sel[:],
                    op0=Alu.mult, op1=Alu.mult,
                )
                nc.tensor.matmul(out=den8, lhsT=pe_t[:], rhs=ones[:],
                                 start=True, stop=True)
                den8_sb = wpool.tile([NG * BL, 1], f32, tag="den8sb")
                nc.scalar.copy(out=den8_sb[:], in_=den8)
                nc.tensor.matmul(out=dsum, lhsT=sel2[:], rhs=den8_sb[:],
                                 start=True, stop=True)
                rden = wpool.tile([BL, 1], f32, tag="rden")
                nc.vector.reciprocal(out=rden[:], in_=dsum)
                # o[b, d] = sum_slots pe_t[slot, b] * E_{hop+1}[slot, d]
                for g in range(NG):
                    nc.tensor.matmul(
                        out=o_ps, lhsT=pe_t[:, 2 * g:2 * g + 2],
                        rhs=E_all[:, g * DCAT + hop * D: g * DCAT + hop * D + D],
                        start=(g == 0), stop=(g == NG - 1),
                    )
                u2 = wpool.tile([BL, D], f32, tag=f"u{hop}")
                nc.vector.scalar_tensor_tensor(
                    out=u2[:], in0=o_ps, scalar=rden[:], in1=u[:],
                    op0=Alu.mult, op1=Alu.add,
                )
                u = u2
                if hop < HOPS - 1:
                    nc.tensor.matmul(out=ucps, lhsT=u[:], rhs=i2[:],
                                     start=True, stop=True)
                    uc = wpool.tile([P, BL], f32, tag=f"uc{hop}")
                    nc.scalar.copy(out=uc[:], in_=ucps)

            nc.sync.dma_start(out=out_d[:], in_=u[:])
    if do_compile:
        nc.compile()
    return nc


def _wrap16(idx):
    """flat [n] int16 -> SBUF layout [128, n//16]: value i at [i%16, i//16],
    replicated to the 8 16-partition groups the Q7 cores read."""
    n = idx.shape[0]
    w = np.zeros((16, n // 16), np.int16)
    w[np.arange(n) % 16, np.arange(n) // 16] = idx
    return np.tile(w, (8, 1))


def prep_inputs(story, C):
    """Host-side: fused fp16 table, sorted/balanced per-core index layouts."""
    story = np.asarray(story)
    C = np.asarray(C, dtype=np.float32)
    s = story.transpose(1, 0, 2).astype(np.int32)       # (B, M, S)
    ccat = np.zeros((V + 1, DCAT), np.float16)
    ccat[:V] = np.concatenate([C[1], C[2], C[3]], axis=1).astype(np.float16)

    # per core: sort tokens in each sentence (low vocab first) and balance
    # sentences across the NG groups by nlow, mixing batch rows freely; the
    # uploaded sel tensor encodes each sentence's batch-row ownership
    sorted_toks = []                                     # [core][g] -> (P, S)
    nlows = []                                           # [core][g] -> (P,)
    owners = []                                          # [core][g] -> (P,) batch row
    for i in range(NCORES):
        blk = s[i * BL:(i + 1) * BL].reshape(NS, S)      # (NS, S)
        own = np.repeat(np.arange(BL), M)                # (NS,)
        nlow = (blk < VSPLIT).sum(1)
        order = np.argsort(nlow, kind="stable")
        st_c, nl_c, ow_c = [], [], []
        for q in ((0, 3, 1, 2) if NG == 4 else range(NG)):
            pick = order[q * P:(q + 1) * P]
            st_c.append(np.sort(blk[pick], axis=1))
            nl_c.append(nlow[pick])
            ow_c.append(own[pick])
        sorted_toks.append(st_c)
        nlows.append(nl_c)
        owners.append(ow_c)

    KA = tuple(int(max(nlows[i][g].max() for i in range(NCORES))) for g in range(NG))
    KB = tuple(S - int(min(nlows[i][g].min() for i in range(NCORES))) for g in range(NG))

    identg = np.eye(P, dtype=np.float16)
    in_maps = []
    for i in range(NCORES):
        m = {"ccat": ccat, "identg": identg}
        blob = np.zeros((P, BLOBC), np.float32)
        blob[:, OFF_IDENT:OFF_IDENT + P] = np.eye(P, dtype=np.float32)
        for g in range(NG):
            blob[np.arange(P), OFF_SEL + g * 2 + owners[i][g]] = 1.0
        blob[0:BL, OFF_I2:OFF_I2 + BL] = np.eye(BL, dtype=np.float32)
        blob[:, OFF_ONES] = 1.0
        for g in range(NG):
            for b in range(BL):
                blob[g * 2 + b, OFF_SEL2 + b] = 1.0
        m["blob"] = blob
        for g in range(NG):
            toks = sorted_toks[i][g]                     # (P, S) sorted
            nlow = nlows[i][g]                           # (P,)
            ka, kb = KA[g], KB[g]
            ks = np.arange(ka)[:, None]                  # slot k -> row k*128+p
            low = np.where(ks < nlow[None, :], toks.T[:ka], 0).astype(np.int16)
            k0 = S - kb
            ksb = (k0 + np.arange(kb))[:, None]
            high = np.where(
                ksb >= nlow[None, :],
                toks.T[k0:].astype(np.int64) - VSPLIT,
                ZHIGH,
            ).astype(np.int16)
            m[f"idx{g}"] = np.concatenate(
                [_wrap16(low.reshape(-1)), _wrap16(high.reshape(-1))], axis=1)
        in_maps.append(m)
    return in_maps, KA, KB


def run(in_maps, KA, KB, trace=False, **kwargs):
    from concourse.bass_utils import run_bass_kernel_spmd

    key = (KA, KB)
    if key not in _CACHE:
        _CACHE[key] = build(KA, KB)
    nc = _CACHE[key]
    res = run_bass_kernel_spmd(
        nc, in_maps, core_ids=list(range(NCORES)), trace=trace, **kwargs
    )
    out = np.concatenate([r["out"] for r in res.results], axis=0)
    return out, res


def kernel(story, C):
    in_maps, KA, KB = prep_inputs(story, C)
    out, _ = run(in_maps, KA, KB)
    return out.astype(np.float32)


# revision 9
# speedup vs baseline: 1.5640x; 1.2197x over previous
"""Trainium2 Bass kernel for nn_EncoderMemNN_14929306321427 (MemNN encoder).

Math (see reference.py): story (M=256, B=16, S=64) token ids; C (4, V, 128)
embedding tables. Per hop h: m_A = sum_S C[h][s], prob = softmax_M(m_A @ u),
m_C = sum_S C[h+1][s], u += prob @ m_C. u starts at 0, so hop-0's softmax is
uniform: C[0] is never needed and u after hop 0 is mean_M(E1).

Strategy: data-parallel over batch (2 rows/core, 8 cores, no collectives).
Host fuses tables 1..3 into ccat[V+1, 384] fp16 (row V = 0) so each token is
ONE 768B dma_gather row. dma_gather indices are int16, so tokens are split at
32768: call A gathers low tokens from the table base, call B gathers high
tokens from a +32768 row view; slots not owned by a call point at an all-zero
row (PAD row 0 / appended row V), which adds 0 to the sum. Tokens are sorted
within each sentence and sentences are nlow-balanced across groups so the two
calls cover disjoint near-minimal slot ranges.

v2: gathers are split into ~24-slot chunks issued round-robin across 4 SWDGE
queues so the DMA engines stay continuously fed and the PE consumes chunks as
they land (instead of 8 big serialized calls). The attention tail keeps all
softmax state slot-major (logits computed transposed via F = E^T), which
replaces the per-group transpose->copy->matmul chains with a handful of tiny
matmuls: exp -> mask -> den (ones-matmul) -> weighted sum, ~2x3us instead of
~2x11us. The hop-0 state u1 = mean_M(E1) is accumulated per group in both
row- and column-major in parallel.
"""

import numpy as np

HOPS = 3
V = 50257
D = 128
M = 256
B = 16
S = 64
NCORES = 8
BL = B // NCORES            # batch rows per core
NS = BL * M                 # sentences per core
P = 128
NG = NS // P                # sentence groups of 128
DCAT = HOPS * D             # 384 = fused row [C1|C2|C3]
VSPLIT = 32768
ZHIGH = V - VSPLIT          # index of appended zero row within the high view
CH = 12                     # max gather-chunk slots
NQ = 4                      # SWDGE queues
GB = 10                     # gather chunk buffers in flight

# const blob column offsets (f32, [128, BLOBC])
OFF_IDENT = 0
OFF_SEL = 128               # sel[p, g*2+b] = 1 if sentence p of group g owned by b
OFF_I2 = 136                # 2x2 identity in rows 0..1
OFF_ONES = 138              # all-ones column
OFF_SEL2 = 139              # sel2[g*2+b, b] = 1 in rows 0..7
BLOBC = 141

_CACHE = {}


def _chunk_plan(k):
    """Split k slots into balanced chunks of at most CH."""
    if k == 0:
        return []
    n = -(-k // CH)
    base, rem = divmod(k, n)
    sizes = [base + (1 if i < rem else 0) for i in range(n)]
    out, off = [], 0
    for s in sizes:
        out.append((off, s))
        off += s
    return out


def build(KA, KB, do_compile=True):
    """KA/KB: per-group slot counts for the low/high gather calls."""
    from concourse import bacc, mybir, tile

    f32 = mybir.dt.float32
    f16 = mybir.dt.float16
    i16 = mybir.dt.int16
    Alu = mybir.AluOpType
    Act = mybir.ActivationFunctionType

    nc = bacc.Bacc(num_swdge_queues=NQ, dynamic_dma_scratch_size=49152)
    ccat_d = nc.declare_dram_parameter("ccat", [V + 1, DCAT], f16, isOutput=False)
    idx_d = [
        nc.declare_dram_parameter(
            f"idx{g}", [P, 8 * (KA[g] + KB[g])], i16, isOutput=False)
        for g in range(NG)
    ]
    blob_d = nc.declare_dram_parameter("blob", [P, BLOBC], f32, isOutput=False)
    identg_d = nc.declare_dram_parameter("identg", [P, P], f16, isOutput=False)
    out_d = nc.declare_dram_parameter("out", [BL, D], f32, isOutput=True)

    with tile.TileContext(nc) as tc:
        with (
            tc.tile_pool(name="const", bufs=1) as cpool,
            tc.tile_pool(name="gather", bufs=GB) as gpool,
            tc.tile_pool(name="work", bufs=2) as wpool,
            tc.tile_pool(name="ps_e", bufs=2, space="PSUM") as ps_e,
            tc.tile_pool(name="ps_t", bufs=2, space="PSUM") as ps_t,
            tc.tile_pool(name="ps_us", bufs=1, space="PSUM") as ps_us,
            tc.tile_pool(name="ps_at", bufs=2, space="PSUM") as ps_at_pool,
        ):
            # priming gathers: the first dma_gather on real HW stalls the
            # Pool engine ~20us (cold ucode) with all queue drains blocked
            # behind it; absorb that during the preamble with tiny gathers
            # of the PAD row on each queue (indices memset to 0, no uploads)
            pidx = cpool.tile([P, 8], i16)
            nc.vector.memset(pidx[:], 0)
            for q in range(NQ):
                pg = cpool.tile([P, 1, DCAT], f16, tag=f"prime{q}")
                nc.gpsimd.dma_gather(
                    out_ap=pg[:], in_ap=ccat_d[:], idxs_ap=pidx[:],
                    num_idxs=P, num_idxs_reg=P, elem_size=DCAT,
                    single_packet=False, queue_num=q,
                )
            idx_sb = []
            for g in range(NG):
                t = cpool.tile(list(idx_d[g].shape), i16, tag=f"idx{g}")
                nc.sync.dma_start(out=t[:], in_=idx_d[g][:])
                idx_sb.append(t)
            blob = cpool.tile([P, BLOBC], f32)
            nc.sync.dma_start(out=blob[:], in_=blob_d[:])
            identg = cpool.tile([P, P], f16)
            nc.sync.dma_start(out=identg[:], in_=identg_d[:])

            ident = blob[:, OFF_IDENT:OFF_IDENT + P]
            sel = blob[:, OFF_SEL:OFF_SEL + NG * BL]
            i2 = blob[0:BL, OFF_I2:OFF_I2 + BL]
            ones = blob[:, OFF_ONES:OFF_ONES + 1]
            sel2 = blob[0:NG * BL, OFF_SEL2:OFF_SEL2 + BL]

            # ---- gather + sentence-sum: E_all[p, g*DCAT+d] = sum_S ccat[tok]
            # plus per group: F1/F2 = E1^T/E2^T and u1 accumulation both ways
            E_all = cpool.tile([P, NG * DCAT], f32)
            F1 = cpool.tile([P, NS], f32)
            F2 = cpool.tile([P, NS], f32)
            # two PSUM banks for the u1 accumulators (one open accumulation
            # chain per bank); their slack space is reused by the attention
            # phase for the tiny den/uc tiles once u1 has been consumed
            usbk = ps_us.tile([P, 512], f32, tag="us")
            ustbk = ps_us.tile([P, 512], f32, tag="ust")
            us_ps = usbk[0:BL, 0:D]
            ust_ps = ustbk[:, 0:BL]
            qctr = 0
            for g in range(NG):
                eps = ps_e.tile([P, DCAT], f32, tag="eacc")
                tot = KA[g] + KB[g]
                nmm = 0
                for K, base_off, src in (
                    (KA[g], 0, ccat_d[:]),
                    (KB[g], 8 * KA[g], ccat_d[VSPLIT:, :]),
                ):
                    for off, csz in _chunk_plan(K):
                        gt = gpool.tile([P, CH, DCAT], f16, tag="ch")
                        nc.gpsimd.dma_gather(
                            out_ap=gt[:, :csz, :], in_ap=src,
                            idxs_ap=idx_sb[g][:, base_off + 8 * off:
                                              base_off + 8 * (off + csz)],
                            num_idxs=P * csz, num_idxs_reg=P * csz,
                            elem_size=DCAT, single_packet=False,
                            queue_num=qctr % NQ,
                        )
                        qctr += 1
                        for r in range(csz):
                            nc.tensor.matmul(
                                out=eps[:], lhsT=identg[:], rhs=gt[:, r, :],
                                start=(nmm == 0), stop=(nmm == tot - 1),
                            )
                            nmm += 1
                nc.vector.tensor_copy(out=E_all[:, g * DCAT:(g + 1) * DCAT], in_=eps[:])
                for t, F in ((0, F1), (1, F2)):
                    tp = ps_t.tile([P, P], f32, tag="tp")
                    nc.tensor.transpose(
                        out=tp[:],
                        in_=E_all[:, g * DCAT + t * D: g * DCAT + t * D + D],
                        identity=ident,
                    )
                    nc.scalar.copy(out=F[:, g * P:(g + 1) * P], in_=tp[:])
                # hop 0: u1 = mean_M E1 (softmax of zero logits is uniform),
                # accumulated row-major (us) and col-major (ust) in parallel
                nc.tensor.matmul(
                    out=us_ps[:], lhsT=sel[:, 2 * g:2 * g + 2],
                    rhs=E_all[:, g * DCAT:g * DCAT + D],
                    start=(g == 0), stop=(g == NG - 1),
                )
                nc.tensor.matmul(
                    out=ust_ps[:], lhsT=E_all[:, g * DCAT:g * DCAT + D],
                    rhs=sel[:, 2 * g:2 * g + 2],
                    start=(g == 0), stop=(g == NG - 1),
                )

            u = wpool.tile([BL, D], f32, tag="u0")
            nc.scalar.activation(out=u[:], in_=us_ps[:], func=Act.Copy, scale=1.0 / M)
            uc = wpool.tile([P, BL], f32, tag="uc0")
            nc.scalar.activation(out=uc[:], in_=ust_ps[:], func=Act.Copy, scale=1.0 / M)

            # ---- hops 1..2, all slot-major: logits lgT[slot, g*2+b]
            for hop in (1, 2):
                F = F1 if hop == 1 else F2
                # per-hop PSUM bank for logits + o; tiny tiles live in the
                # retired u1 banks (each bank has one open chain at a time)
                at = ps_at_pool.tile([P, 512], f32, tag="at")
                lgT = at[:, 0:NG * BL]
                o_ps = at[0:BL, 16:16 + D]
                den8 = ustbk[0:NG * BL, 100 + 10 * hop:101 + 10 * hop]
                dsum = usbk[0:BL, 200 + 10 * hop:201 + 10 * hop]
                ucps = ustbk[:, 200 + 10 * hop:202 + 10 * hop]
                for g in range(NG):
                    nc.tensor.matmul(
                        out=lgT[:, 2 * g:2 * g + 2], lhsT=F[:, g * P:(g + 1) * P],
                        rhs=uc[:], start=True, stop=True,
                    )
                # exp (no max-sub: |logits| <~ 6 by construction), then mask
                # to the owning batch row (sel is exactly the 0/1 ownership)
                pe_raw = wpool.tile([P, NG * BL], f32, tag="praw")
                nc.scalar.activation(out=pe_raw[:], in_=lgT, func=Act.Exp)
                pe_t = wpool.tile([P, NG * BL], f32, tag="pet")
                nc.vector.scalar_tensor_tensor(
                    out=pe_t[:], in0=pe_raw[:], scalar=1.0, in1=sel[:],
                    op0=Alu.mult, op1=Alu.mult,
                )
                nc.tensor.matmul(out=den8, lhsT=pe_t[:], rhs=ones[:],
                                 start=True, stop=True)
                den8_sb = wpool.tile([NG * BL, 1], f32, tag="den8sb")
                nc.scalar.copy(out=den8_sb[:], in_=den8)
                nc.tensor.matmul(out=dsum, lhsT=sel2[:], rhs=den8_sb[:],
                                 start=True, stop=True)
                rden = wpool.tile([BL, 1], f32, tag="rden")
                nc.vector.reciprocal(out=rden[:], in_=dsum)
                # o[b, d] = sum_slots pe_t[slot, b] * E_{hop+1}[slot, d]
                for g in range(NG):
                    nc.tensor.matmul(
                        out=o_ps, lhsT=pe_t[:, 2 * g:2 * g + 2],
                        rhs=E_all[:, g * DCAT + hop * D: g * DCAT + hop * D + D],
                        start=(g == 0), stop=(g == NG - 1),
                    )
                u2 = wpool.tile([BL, D], f32, tag=f"u{hop}")
                nc.vector.scalar_tensor_tensor(
                    out=u2[:], in0=o_ps, scalar=rden[:], in1=u[:],
                    op0=Alu.mult, op1=Alu.add,
                )
                u = u2
                if hop < HOPS - 1:
                    nc.tensor.matmul(out=ucps, lhsT=u[:], rhs=i2[:],
                                     start=True, stop=True)
                    uc = wpool.tile([P, BL], f32, tag=f"uc{hop}")
                    nc.scalar.copy(out=uc[:], in_=ucps)

            nc.sync.dma_start(out=out_d[:], in_=u[:])
    if do_compile:
        nc.compile()
    return nc


def _wrap16(idx):
    """flat [n] int16 -> SBUF layout [128, n//16]: value i at [i%16, i//16],
    replicated to the 8 16-partition groups the Q7 cores read."""
    n = idx.shape[0]
    w = np.zeros((16, n // 16), np.int16)
    w[np.arange(n) % 16, np.arange(n) // 16] = idx
    return np.tile(w, (8, 1))


def prep_inputs(story, C):
    """Host-side: fused fp16 table, sorted/balanced per-core index layouts."""
    story = np.asarray(story)
    C = np.asarray(C, dtype=np.float32)
    s = story.transpose(1, 0, 2).astype(np.int32)       # (B, M, S)
    ccat = np.zeros((V + 1, DCAT), np.float16)
    ccat[:V] = np.concatenate([C[1], C[2], C[3]], axis=1).astype(np.float16)

    # per core: sort tokens in each sentence (low vocab first) and balance
    # sentences across the NG groups by nlow, mixing batch rows freely; the
    # uploaded sel tensor encodes each sentence's batch-row ownership
    sorted_toks = []                                     # [core][g] -> (P, S)
    nlows = []                                           # [core][g] -> (P,)
    owners = []                                          # [core][g] -> (P,) batch row
    for i in range(NCORES):
        blk = s[i * BL:(i + 1) * BL].reshape(NS, S)      # (NS, S)
        own = np.repeat(np.arange(BL), M)                # (NS,)
        nlow = (blk < VSPLIT).sum(1)
        order = np.argsort(nlow, kind="stable")
        st_c, nl_c, ow_c = [], [], []
        for q in ((0, 3, 1, 2) if NG == 4 else range(NG)):
            pick = order[q * P:(q + 1) * P]
            st_c.append(np.sort(blk[pick], axis=1))
            nl_c.append(nlow[pick])
            ow_c.append(own[pick])
        sorted_toks.append(st_c)
        nlows.append(nl_c)
        owners.append(ow_c)

    KA = tuple(int(max(nlows[i][g].max() for i in range(NCORES))) for g in range(NG))
    KB = tuple(S - int(min(nlows[i][g].min() for i in range(NCORES))) for g in range(NG))

    identg = np.eye(P, dtype=np.float16)
    in_maps = []
    for i in range(NCORES):
        m = {"ccat": ccat, "identg": identg}
        blob = np.zeros((P, BLOBC), np.float32)
        blob[:, OFF_IDENT:OFF_IDENT + P] = np.eye(P, dtype=np.float32)
        for g in range(NG):
            blob[np.arange(P), OFF_SEL + g * 2 + owners[i][g]] = 1.0
        blob[0:BL, OFF_I2:OFF_I2 + BL] = np.eye(BL, dtype=np.float32)
        blob[:, OFF_ONES] = 1.0
        for g in range(NG):
            for b in range(BL):
                blob[g * 2 + b, OFF_SEL2 + b] = 1.0
        m["blob"] = blob
        for g in range(NG):
            toks = sorted_toks[i][g]                     # (P, S) sorted
            nlow = nlows[i][g]                           # (P,)
            ka, kb = KA[g], KB[g]
            ks = np.arange(ka)[:, None]                  # slot k -> row k*128+p
            low = np.where(ks < nlow[None, :], toks.T[:ka], 0).astype(np.int16)
            k0 = S - kb
            ksb = (k0 + np.arange(kb))[:, None]
            high = np.where(
                ksb >= nlow[None, :],
                toks.T[k0:].astype(np.int64) - VSPLIT,
                ZHIGH,
            ).astype(np.int16)
            m[f"idx{g}"] = np.concatenate(
                [_wrap16(low.reshape(-1)), _wrap16(high.reshape(-1))], axis=1)
        in_maps.append(m)
    return in_maps, KA, KB


def run(in_maps, KA, KB, trace=False, **kwargs):
    from concourse.bass_utils import run_bass_kernel_spmd

    key = (KA, KB)
    if key not in _CACHE:
        _CACHE[key] = build(KA, KB)
    nc = _CACHE[key]
    res = run_bass_kernel_spmd(
        nc, in_maps, core_ids=list(range(NCORES)), trace=trace, **kwargs
    )
    out = np.concatenate([r["out"] for r in res.results], axis=0)
    return out, res


def kernel(story, C):
    in_maps, KA, KB = prep_inputs(story, C)
    out, _ = run(in_maps, KA, KB)
    return out.astype(np.float32)
